# revision 6
# baseline (speedup 1.0000x reference)
# GATConv Trainium kernel: host prep + Bass program builder (parameterized).
import numpy as np
import ml_dtypes
import concourse.bass as bass
import concourse.bacc as bacc
import concourse.mybir as mybir
import concourse.tile as tile
from concourse._compat import exact_div

F32 = mybir.dt.float32
BF16 = mybir.dt.bfloat16
I16 = mybir.dt.int16

ALPHA = 0.2
H, D = 8, 32
HD = H * D            # 256
IN = 256
ROW = HD + 16         # ft row: 256 ft | 8 el | 8 er (all bf16) = 272 cols
FT_W = 384            # padded ft row (bf16) -> 768B stride
WMAX = 8              # max blocks (128 idx each) per gather window


def _ceil(a, b):
    return -(-a // b)


class Plan:
    """Host-side uniform schedule shared by all cores."""

    def __init__(self, N, E, src, dst, n_cores, tiles_per_core, st_tiles=6, chunk=32768):
        self.N, self.E, self.C = N, E, n_cores
        self.NT = tiles_per_core              # dst tiles per core
        self.ND = tiles_per_core * 128        # dsts per core
        self.NN = self.ND * n_cores           # padded node count (tables)
        assert self.NN >= N
        self.chunk = chunk
        self.NQ = _ceil(self.NN, chunk)       # chunks
        self.ST = st_tiles

        order = np.argsort(dst, kind="stable")
        src_s, dst_s = src[order], dst[order]
        core_of = dst_s // self.ND
        tile_of = (dst_s % self.ND) // 128
        # rotated src coordinate per edge: (src - core*ND) mod NN
        rot_src = (src_s - core_of * self.ND) % self.NN
        q_of = rot_src // chunk

        cnt = np.zeros((n_cores, self.NT, self.NQ), dtype=np.int64)
        np.add.at(cnt, (core_of, tile_of, q_of), 1)
        B = _ceil(cnt, 128).max(axis=0)       # [NT, NQ] blocks per group
        B[:, 0] = np.maximum(B[:, 0], 1)      # every tile has >=1 block
        self.B = B
        self.blocks_per_tile = B.sum(axis=1)  # [NT]

        # per-(c,t,q) edge lists
        self.edges = [[[None] * self.NQ for _ in range(self.NT)] for _ in range(n_cores)]
        key = ((core_of * self.NT + tile_of) * self.NQ + q_of)
        order2 = np.argsort(key, kind="stable")
        ks = key[order2]
        bounds = np.searchsorted(ks, np.arange(n_cores * self.NT * self.NQ + 1))
        for c in range(n_cores):
            for t in range(self.NT):
                for q in range(self.NQ):
                    k = (c * self.NT + t) * self.NQ + q
                    sel = order2[bounds[k]:bounds[k + 1]]
                    self.edges[c][t][q] = (rot_src[sel], dst_s[sel])

        # call schedule: supertile-major, chunk-minor
        self.n_st = _ceil(self.NT, st_tiles)
        self.calls = []                       # (q, [(t, B_tq), ...])
        for s in range(self.n_st):
            ts = range(s * st_tiles, min((s + 1) * st_tiles, self.NT))
            for q in range(self.NQ):
                items = [(t, int(B[t, q])) for t in ts if B[t, q] > 0]
                if items:
                    self.calls.append((q, items))
        self.NBtot = int(B.sum() * 128)

        # per-call stream column map (i16 cols): [ift | ier | dstl(f32 raw)]
        # per call with nb blocks: nb*8 + nb*8 + nb*2 = nb*18 cols
        self.call_cols = []
        c0 = 0
        for q, items in self.calls:
            nb = sum(b for _, b in items)
            self.call_cols.append((c0, nb))
            c0 += nb * 18
        self.SC = c0

    def build_streams(self, c):
        stream = np.zeros((128, self.SC), dtype=np.int16)
        i128 = np.arange(128)
        for (q, items), (c0, nb) in zip(self.calls, self.call_cols):
            NBc = nb * 128
            idx_ft = np.zeros(NBc, dtype=np.int16)
            idx_er = np.zeros(NBc, dtype=np.int16)
            dstl = np.full(NBc, 200.0, dtype=np.float32)
            pos = 0
            for t, nbt in items:
                s_arr, d_arr = self.edges[c][t][q]
                n = len(s_arr)
                assert n <= nbt * 128
                assert (s_arr >= q * self.chunk).all() and (s_arr < (q + 1) * self.chunk).all()
                idx_ft[pos:pos + n] = (s_arr - q * self.chunk).astype(np.int16)
                idx_er[pos:pos + n] = (d_arr - c * self.ND).astype(np.int16)
                dstl[pos:pos + n] = (d_arr - (c * self.ND + t * 128)).astype(np.float32)
                pos += nbt * 128
            assert pos == NBc
            # gather idx format: [128, n/16], 16-partition wrap, x8 replicated
            i = np.arange(NBc)
            ift = np.zeros((128, NBc // 16), dtype=np.int16)
            ier = np.zeros((128, NBc // 16), dtype=np.int16)
            for k in range(8):
                ift[16 * k + i % 16, i // 16] = idx_ft
                ier[16 * k + i % 16, i // 16] = idx_er
            dl = np.zeros((128, nb), dtype=np.float32)
            dl[i % 128, i // 128] = dstl
            stream[:, c0:c0 + nb * 8] = ift
            stream[:, c0 + nb * 8:c0 + nb * 16] = ier
            stream[:, c0 + nb * 16:c0 + nb * 18] = (
                np.ascontiguousarray(dl).view('<i2'))
        return stream


def make_waug(W, attn_l, attn_r):
    """[IN, 272] f32 cols: [W'^T | Ml | Mr]; W' rows in d-major order d*H+h."""
    perm = np.empty(HD, dtype=np.int64)
    for h in range(H):
        for d in range(D):
            perm[d * H + h] = h * D + d
    Wp = W[perm, :]                                   # [256, IN]
    Ml = np.zeros((IN, H), dtype=np.float32)
    Mr = np.zeros((IN, H), dtype=np.float32)
    for h in range(H):
        rows = W[h * D:(h + 1) * D, :]                # [D, IN]
        Ml[:, h] = attn_l[0, h, :] @ rows
        Mr[:, h] = attn_r[0, h, :] @ rows
    return np.concatenate([Wp.T, Ml, Mr], axis=1).astype(np.float32)


def dma_gather_raw(gp, out_ap, in_ap, idxs_ap, num_idxs, elem_size, elem_step,
                   queue_num=0):
    """dma_gather minus the elem_size%256 assert (row stride must be %256B)."""
    stride_bytes = elem_step * mybir.dt.size(in_ap.dtype)
    stride_bytes_256 = exact_div(stride_bytes, 256)
    _in_ap = gp.lower_ap_dma(in_ap, for_custom_bir_dma=True)
    _idxs_ap = gp.lower_ap(idxs_ap)
    _out_ap = gp.lower_ap(out_ap)
    return gp.add_instruction(
        mybir.InstDMAGatherAnt(
            name=gp.bass.get_next_instruction_name(),
            ins=[*_in_ap, _idxs_ap, gp.lower_val_access(gp.to_reg(num_idxs))],
            outs=[_out_ap],
            transpose=False, num_idxs=num_idxs, elem_size=elem_size,
            stride_bytes_256=stride_bytes_256, gen_mode=0, single_packet=True,
            queue_num=queue_num, sbuf_tokens_per_rank=0, sbuf_free_dim_per_rank=0,
            sbuf_free_dim_pad_per_rank=0, sbuf_byte_offset=0,
        )
    )


def build_program(plan, n_cores, fc_mega=16, stages=5, nq=2):
    """One SPMD Bass program. Inputs: featT bf16 [IN,NN], waug bf16 [IN,272],
    stream i16 [128, SC], iota bf16. Output: out [ND, 256] f32 (h-major cols)."""
    p = plan
    NN, ND, NT = p.NN, p.ND, p.NT
    nc = bacc.Bacc("TRN2", target_bir_lowering=False, debug=False,
                   num_devices=n_cores, num_swdge_queues=nq,
                   dynamic_dma_scratch_size=16384 * (2 if WMAX > 8 else 1))

    featT_d = nc.dram_tensor("featT", [IN, NN], BF16, kind="ExternalInput").ap()
    waug_d = nc.dram_tensor("waug", [IN, ROW], BF16, kind="ExternalInput").ap()
    stream_d = nc.dram_tensor("stream", [128, p.SC], I16, kind="ExternalInput").ap()
    iota_d = nc.dram_tensor("iota", [128, 128], BF16, kind="ExternalInput").ap()
    ft_t = nc.dram_tensor("ft_tab", [NN, FT_W], BF16, kind="Internal").ap()
    out_d = nc.dram_tensor("out", [ND, HD], F32, kind="ExternalOutput").ap()

    n_nt = NN // 128
    MG = fc_mega

    with tile.TileContext(nc) as tc:
        # ---------------- Phase A: FC over all nodes ----------------
        with tc.tile_pool(name="fca", bufs=2) as pool, \
             tc.tile_pool(name="fcc", bufs=1) as cpool, \
             tc.tile_pool(name="fcp", bufs=2, space="PSUM") as psp:
            wa = cpool.tile([128, 2, ROW], BF16)
            nc.sync.dma_start(wa[:], waug_d.rearrange("(k p) c -> p k c", p=128))
            for g0 in range(0, n_nt, MG):
                gn = min(MG, n_nt - g0)
                ftin = pool.tile([128, 2, MG * 128], BF16, tag="ftin")
                nc.sync.dma_start(
                    ftin[:, :, :gn * 128],
                    featT_d.rearrange("(k p) n -> p k n", p=128)[:, :, g0 * 128:(g0 + gn) * 128])
                ftst = pool.tile([128, MG, ROW], BF16, tag="ftst")
                for j0 in range(0, gn, 2):
                    jn = min(2, gn - j0)
                    # 512-col stride: each jj's matmul output stays in its own
                    # PSUM bank (272 f32 would cross the 2KB bank boundary)
                    fc_ps = psp.tile([128, 2, 512], F32, tag="fc")
                    for jj in range(jn):
                        for k in range(2):
                            nc.tensor.matmul(fc_ps[:, jj, 0:ROW],
                                             ftin[:, k, (j0 + jj) * 128:(j0 + jj + 1) * 128],
                                             wa[:, k, :], start=(k == 0), stop=(k == 1))
                    nc.any.tensor_copy(ftst[:, j0:j0 + jn, :], fc_ps[:, :jn, 0:ROW])
                nc.sync.dma_start(
                    ft_t.rearrange("(g p) c -> p g c", p=128)[:, g0:g0 + gn, 0:ROW],
                    ftst[:, :gn, :])

        # ---------------- Phase B: edge pipeline ----------------
        with tc.tile_pool(name="ebc", bufs=1) as cpool, \
             tc.tile_pool(name="ebs", bufs=2) as spool, \
             tc.tile_pool(name="eb", bufs=3) as pool, \
             tc.tile_pool(name="oh", bufs=4) as ohpool, \
             tc.tile_pool(name="ebo", bufs=2) as opool, \
             tc.tile_pool(name="ebp", bufs=6, space="PSUM") as psp:
            iota_row = cpool.tile([128, 128], BF16)
            nc.sync.dma_start(iota_row[:], iota_d[:])
            agg = {}
            issued = {t: 0 for t in range(NT)}
            ost = {}
            wcount = 0
            calls = p.calls if stages >= 2 else []
            smax = max(nb for _, nb in p.call_cols) if p.call_cols else 1
            for ci, (q, items) in enumerate(calls):
                c0, nbc = p.call_cols[ci]
                st = spool.tile([128, 18 * smax], I16, tag="st")
                nc.sync.dma_start(st[:, :nbc * 18], stream_d[:, c0:c0 + nbc * 18])
                blocks = []                     # flat tile-id per block
                for t, nbt in items:
                    blocks += [t] * nbt
                hi = min((q + 1) * p.chunk, NN)
                w0 = 0
                while w0 < len(blocks):
                    wn = min(WMAX, len(blocks) - w0)
                    NB = wn * 128
                    g = pool.tile([128, WMAX, ROW], BF16, tag="g")
                    erg = pool.tile([128, WMAX, H], BF16, tag="erg")
                    if stages >= 2:
                        dma_gather_raw(nc.gpsimd, g[:, :wn, :],
                                       ft_t[q * p.chunk:hi, 0:ROW],
                                       st[:, w0 * 8:w0 * 8 + NB // 16],
                                       NB, ROW, FT_W, queue_num=0)
                        dma_gather_raw(nc.gpsimd, erg[:, :wn, :],
                                       ft_t[0:ND, HD + 8:HD + 16],
                                       st[:, nbc * 8 + w0 * 8:nbc * 8 + w0 * 8 + NB // 16],
                                       NB, H, FT_W, queue_num=nq - 1)
                    dstl = st[:, nbc * 16 + w0 * 2:nbc * 16 + (w0 + wn) * 2].bitcast(F32)
                    if stages >= 3:
                        lw = pool.tile([128, WMAX, H], BF16, tag="lw")
                        nc.vector.tensor_tensor(lw[:, :wn, :], g[:, :wn, HD:HD + 8],
                                                erg[:, :wn, :], mybir.AluOpType.add)
                        nc.vector.scalar_tensor_tensor(lw[:, :wn, :], lw[:, :wn, :],
                                                       ALPHA, lw[:, :wn, :],
                                                       mybir.AluOpType.mult,
                                                       mybir.AluOpType.max)
                        ee = pool.tile([128, WMAX, H], BF16, tag="ee")
                        nc.scalar.activation(ee[:, :wn, :], lw[:, :wn, :],
                                             mybir.ActivationFunctionType.Exp)
                    if stages >= 4:
                        rhs = pool.tile([128, WMAX, HD + 8], BF16, tag="rhs")
                        nc.vector.tensor_tensor(
                            rhs[:, :wn, 0:HD].rearrange("p b (d h) -> p b d h", h=H),
                            g[:, :wn, 0:HD].rearrange("p b (d h) -> p b d h", h=H),
                            ee[:, :wn, :].unsqueeze(2).broadcast_to([128, wn, D, H]),
                            mybir.AluOpType.mult)
                        nc.any.tensor_copy(rhs[:, :wn, HD:HD + 8], ee[:, :wn, :])
                    if stages >= 5:
                        for j in range(wn):
                            t = blocks[w0 + j]
                            if t not in agg:
                                agg[t] = psp.tile([128, HD + 8], F32, tag="agg",
                                                  name=f"agg{t}")
                            at = agg[t]
                            tot = int(p.blocks_per_tile[t])
                            oh = ohpool.tile([128, 128], BF16, tag="oh")
                            nc.vector.tensor_scalar(oh[:], iota_row[:],
                                                    dstl[:, j:j + 1],
                                                    None, mybir.AluOpType.is_equal)
                            nc.tensor.matmul(at[:], oh[:], rhs[:, j, :],
                                             start=(issued[t] == 0),
                                             stop=(issued[t] == tot - 1),
                                             skip_group_check=True)
                            issued[t] += 1
                            if issued[t] == tot:
                                s = t // p.ST
                                if s not in ost:
                                    ost[s] = opool.tile([128, p.ST, HD], F32,
                                                        tag="ost", name=f"ost{s}")
                                pool_ost = ost[s]
                                dsum = pool.tile([128, H], F32, tag="dsum")
                                nc.vector.tensor_scalar(dsum[:], at[:, HD:HD + 8],
                                                        1e-20, None,
                                                        mybir.AluOpType.max)
                                recd = pool.tile([128, H], F32, tag="recd")
                                nc.vector.reciprocal(recd[:], dsum[:])
                                nc.vector.tensor_tensor(
                                    pool_ost[:, t % p.ST, :].rearrange(
                                        "p (h d) -> p h d", d=D),
                                    at[:, 0:HD].rearrange("p (d h) -> p h d", h=H),
                                    recd[:].unsqueeze(2).broadcast_to([128, H, D]),
                                    mybir.AluOpType.mult)
                                del agg[t]
                                t0 = s * p.ST
                                n_in_st = min(p.ST, NT - t0)
                                if all(issued[tt] == int(p.blocks_per_tile[tt])
                                       for tt in range(t0, t0 + n_in_st)):
                                    nc.sync.dma_start(
                                        out_d.rearrange("(g p) c -> p g c", p=128)[:, t0:t0 + n_in_st, :],
                                        pool_ost[:, :n_in_st, :])
                                    del ost[s]
                    w0 += wn
                    wcount += 1
    return _finish(nc)


def _finish(nc):
    nc.compile()
    return nc


def host_prep(feat, W, attn_l, attn_r, src, dst, n_cores, tiles_per_core,
              st_tiles=6, chunk=32768):
    N = feat.shape[0]
    E = src.shape[0]
    plan = Plan(N, E, src.astype(np.int64), dst.astype(np.int64), n_cores,
                tiles_per_core, st_tiles, chunk)
    featT = np.zeros((IN, plan.NN), dtype=ml_dtypes.bfloat16)
    featT[:, :N] = feat.T.astype(ml_dtypes.bfloat16)
    waug = make_waug(W, attn_l, attn_r).astype(ml_dtypes.bfloat16)
    iota_np = np.broadcast_to(np.arange(128, dtype=np.float32),
                              (128, 128)).astype(ml_dtypes.bfloat16)
    in_maps = []
    for c in range(n_cores):
        rot = np.roll(featT, -c * plan.ND, axis=1)   # col j = node (c*ND+j) mod NN
        in_maps.append({
            "featT": np.ascontiguousarray(rot), "waug": waug,
            "stream": plan.build_streams(c),
            "iota": np.ascontiguousarray(iota_np),
        })
    return plan, in_maps


def assemble_output(plan, results, N):
    full = np.zeros((plan.NN, HD), dtype=np.float32)
    for c in range(plan.C):
        full[c * plan.ND:(c + 1) * plan.ND] = results[c]["out"]
    return full[:N].reshape(N, H, D)


# ----------------------------------------------------------------------------
# Harness entrypoint: full inputs in, full output out. Shapes hardcoded for
# nn_GATConv (N=100000, E=1600000, IN=256, H=8, D=32) on 8 NeuronCores.
# ----------------------------------------------------------------------------
from concourse.bass_interp import get_hw_module as _get_hw_module
from concourse import bass_utils as _bass_utils

_N_CORES = 8
_TPC = 98            # dst tiles per core (98*128*8 = 100352 >= 100000)
_ST_TILES = 4
_CHUNK = 32768
_NQ = 2              # SWDGE queues: ft-gathers and er-gathers in parallel

_cache = {}


def kernel(feat, W, attn_l, attn_r, src, dst):
    feat = np.ascontiguousarray(np.asarray(feat, dtype=np.float32))
    W = np.ascontiguousarray(np.asarray(W, dtype=np.float32))
    attn_l = np.asarray(attn_l, dtype=np.float32)
    attn_r = np.asarray(attn_r, dtype=np.float32)
    src = np.asarray(src).astype(np.int64)
    dst = np.asarray(dst).astype(np.int64)
    N = feat.shape[0]

    plan, in_maps = host_prep(feat, W, attn_l, attn_r, src, dst,
                              _N_CORES, _TPC, st_tiles=_ST_TILES, chunk=_CHUNK)
    key = "prog"
    if key not in _cache:
        nc = build_program(plan, _N_CORES, nq=_NQ)
        nc.m = _get_hw_module(nc.m)
        _cache[key] = nc
    nc = _cache[key]
    res = _bass_utils.run_bass_kernel_spmd(nc, in_maps,
                                           core_ids=list(range(_N_CORES)))
    return assemble_output(plan, res.results, N)
